# revision 1
# baseline (speedup 1.0000x reference)
"""AttentiveGraphConvolution (GAT-style layer) on 8 trn2 NeuronCores.

Math (reference):
    h   = x @ W                       [N, D]
    a_s = h @ attn_self               [N, 1]
    a_n = h @ attn_neigh              [N, 1]
    e   = leaky_relu(a_s + a_n.T, 0.2)
    e   = e + NEG_INF * (1 - adj)
    out = relu(softmax(e, -1) @ h)

Reformulation used here (exact in fp32 up to rounding):
    s_ij = a_s[i] + a_n[j]
    exp(leaky(s)) = exp(0.2 s) * max(exp(0.8 s), 1)       (leaky alpha = 0.2)
    exp(0.8 s)    = w[i] * w2[j],  w = e^{0.8 a_s}, w2 = e^{0.8 a_n}
    adj binary =>  masked weight t_ij = adj_ij * u2_i * v2_j * max(w_i w2_j, 1)

    out_i = relu( (sum_j t_ij h_j) / (sum_j t_ij) )
          = relu( (sum_j q_ji h2_j) / (sum_j q_ji v2_j) )   (u2_i cancels)
    with q_ji  = adjT_ji * max(w_i w2_j, 1)                 [j, i] layout
         h2_j  = v2_j * h_j

Per adj tile the device work is:  R = w2_j * W_bcast  (ACT copy-with-scale),
q = (R max 1) * adjT  (DVE scalar_tensor_tensor), then accumulating float32r
matmuls  outT += h2_chunk.T @ q  and  rs += v2_chunk.T @ q  on the PE.

Changes vs the first-pass kernel (profile-driven):
  * ONE collective instead of two.  Each NRT collective costs ~20 us of
    serial CC-stream time on top of the launch-stagger rendezvous, and the
    trace showed a further 33 us gpsimd stall between the a_n AllGather and
    the h2 AllGather trigger.  a_n now rides in the same gathered tensor as
    h2 (8 extra rows appended to the 1024-row partition-major h2 block).
  * a_s/a_n are computed straight from x via av2 = [W@attn_self|W@attn_neigh]
    (a [2,512] matmul then av = av2T @ x), so h2 production and the gather
    trigger no longer wait for the full hT pass.
  * x and W ship as bf16 (a_v/h accumulate in fp32 PSUM; rel-err stays ~5e-3,
    well inside the 2e-2 gate), halving the phase-1 DMA.
  * The gathered h2 lands in 8 per-source-core tiles so the first matmul
    only waits on block 0's read-back, not all eight.
The main accumulation loop is untouched from the validated baseline.

Sharding: output rows across 8 cores. Each core receives its adj row-slab as
bf16 (adj is binary so bf16 is exact), pre-transposed and row-interleaved in
groups of GP=4 (host layout choice) so each DMA descriptor covers 4 adjacency
rows = 8 KB contiguous.
"""

import numpy as np

N = 8192
DIN = 512
DOUT = 128
NCORES = 8
S = N // NCORES     # 1024 output rows per core
GP = 4              # adjacency rows per partition per DMA (descriptor size)


def _emit(nc, tc, ctx, n, s, din, dout):
    from concourse import masks, mybir

    f32 = mybir.dt.float32
    f32r = mybir.dt.float32r
    bf16 = mybir.dt.bfloat16
    AF = mybir.ActivationFunctionType
    ALU = mybir.AluOpType

    P = 128
    jc_n = n // P       # j chunks over all nodes
    sc_n = s // P       # chunks in the local row slab
    kc_n = din // P     # contraction chunks for x @ W
    nb = min(512, s)    # matmul moving-dim block
    ib_n = s // nb      # i blocks per core (free dim of main matmuls)
    g_n = jc_n // GP    # adj super-chunks (GP j-chunks per DMA)
    sr = s + sc_n       # gathered rows per core: 1024 h2 + 8 a_n rows

    adjt = nc.dram_tensor("adjt", [n, s], bf16, kind="ExternalInput")
    xt = nc.dram_tensor("xt", [din, s], bf16, kind="ExternalInput")
    wmat = nc.dram_tensor("wmat", [din, dout], bf16, kind="ExternalInput")
    wt = nc.dram_tensor("wt", [dout, din], bf16, kind="ExternalInput")
    att = nc.dram_tensor("att", [dout, 2], bf16, kind="ExternalInput")
    out = nc.dram_tensor("out", [s, dout], f32, kind="ExternalOutput")

    const_pool = ctx.enter_context(tc.tile_pool(name="const", bufs=1))
    ph1_pool = ctx.enter_context(tc.tile_pool(name="ph1", bufs=1))
    ph1_psum = ctx.enter_context(tc.tile_pool(name="ph1_psum", bufs=1, space="PSUM"))
    tp_psum = ctx.enter_context(tc.tile_pool(name="tp_psum", bufs=2, space="PSUM"))
    acc_psum = ctx.enter_context(tc.tile_pool(name="acc_psum", bufs=1, space="PSUM"))
    dram_pool = ctx.enter_context(tc.tile_pool(name="dram", bufs=1, space="DRAM"))
    adj_pool = ctx.enter_context(tc.tile_pool(name="adj", bufs=16))
    r_pool = ctx.enter_context(tc.tile_pool(name="r", bufs=5))
    q_pool = ctx.enter_context(tc.tile_pool(name="q", bufs=8))
    fin_pool = ctx.enter_context(tc.tile_pool(name="fin", bufs=2))

    ident = const_pool.tile([P, P], f32, name="ident")
    masks.make_identity(nc, ident[:])
    identb = const_pool.tile([P, P], bf16, name="identb")
    nc.scalar.activation(identb[:], ident[:], AF.Copy)
    identr = const_pool.tile([P, P], f32r, name="identr")
    nc.scalar.activation(identr[:], ident[:], AF.Copy)

    # ---- Phase 1: input DMAs, attention vectors straight from x ----------
    w_sb = []
    x_sb = []
    for k in range(kc_n):
        wk = ph1_pool.tile([P, dout], bf16, name="w_sb", tag=f"w_sb{k}")
        nc.sync.dma_start(wk[:], wmat[k * P:(k + 1) * P, :])
        w_sb.append(wk)
        xk = ph1_pool.tile([P, s], bf16, name="x_sb", tag=f"x_sb{k}")
        nc.sync.dma_start(xk[:], xt[k * P:(k + 1) * P, :])
        x_sb.append(xk)
    wt_sb = ph1_pool.tile([P, din], bf16, name="wt_sb")
    nc.sync.dma_start(wt_sb[:], wt[:])
    att_sb = const_pool.tile([P, 2], bf16, name="att_sb")
    nc.sync.dma_start(att_sb[:], att[:])

    # av2 = [W@attn_self | W@attn_neigh].T : [2, din]
    av2_ps = tp_psum.tile([2, din], f32, name="av2_ps", tag="tp")
    nc.tensor.matmul(av2_ps[:], att_sb[:], wt_sb[:], start=True, stop=True)
    av2_sb = ph1_pool.tile([2, din], bf16, name="av2_sb")
    nc.scalar.activation(av2_sb[:], av2_ps[:], AF.Copy)
    av2T_sb = []
    for k in range(kc_n):
        avT_ps = tp_psum.tile([P, 2], bf16, name="avT_ps", tag="tp")
        nc.tensor.matmul(
            avT_ps[:], av2_sb[:, k * P:(k + 1) * P], identb[:2, :2],
            is_transpose=True, start=True, stop=True,
        )
        a2t = ph1_pool.tile([P, 2], bf16, name="av2T_sb", tag=f"av2T{k}")
        nc.scalar.activation(a2t[:], avT_ps[:], AF.Copy)
        av2T_sb.append(a2t)
    # av[2, s] = [a_s ; a_n] for the local slab, straight from x
    av_sb = ph1_pool.tile([2, s], f32r, name="av_sb")
    for b in range(ib_n):
        avl_ps = tp_psum.tile([2, nb], f32, name="avl_ps", tag="tp")
        for k in range(kc_n):
            nc.tensor.matmul(
                avl_ps[:], av2T_sb[k][:], x_sb[k][:, b * nb:(b + 1) * nb],
                start=(k == 0), stop=(k == kc_n - 1),
            )
        nc.scalar.activation(av_sb[:, b * nb:(b + 1) * nb], avl_ps[:], AF.Copy)

    # W_bcast[p, i] = exp(0.8 * a_s_local[i]) for every partition p
    wrow_sb = ph1_pool.tile([1, s], bf16, name="wrow_sb")
    nc.scalar.activation(wrow_sb[:], av_sb[0:1, :], AF.Exp, scale=0.8)
    ones_sb = const_pool.tile([1, P], bf16, name="ones_sb")
    nc.gpsimd.memset(ones_sb[:], 1.0)
    wb_sb = const_pool.tile([P, s], bf16, name="wb_sb")
    for b in range(ib_n):
        wb_ps = tp_psum.tile([P, nb], f32, name="wb_ps", tag="tp")
        nc.tensor.matmul(
            wb_ps[:], ones_sb[:], wrow_sb[:, b * nb:(b + 1) * nb],
            start=True, stop=True,
        )
        nc.scalar.activation(wb_sb[:, b * nb:(b + 1) * nb], wb_ps[:], AF.Copy)

    # hT[d, n_local] = (x @ W).T for the local slab
    hT_sb = ph1_pool.tile([P, s], f32, name="hT_sb")
    for b in range(ib_n):
        hT_ps = ph1_psum.tile([P, nb], f32, name="hT_ps")
        for k in range(kc_n):
            nc.tensor.matmul(
                hT_ps[:],
                w_sb[k][:],
                x_sb[k][:, b * nb:(b + 1) * nb],
                start=(k == 0), stop=(k == kc_n - 1),
            )
        nc.scalar.activation(hT_sb[:, b * nb:(b + 1) * nb], hT_ps[:], AF.Copy)

    # ---- Phase 2: h2 shard + a_n packed into ONE gathered tensor ---------
    # Local chunk c is written to rows {p*sc_n + c} so the gathered tensor
    # reads back with 4 KB-contiguous per-partition descriptors; rows
    # s..s+sc_n-1 carry a_n for the local slab (row t = nodes t*128..).
    anT_sb = ph1_pool.tile([P, sc_n], f32, name="anT_sb")
    for c in range(sc_n):
        avT2_ps = tp_psum.tile([P, 2], f32r, name="avT2_ps", tag="tp")
        nc.tensor.matmul(
            avT2_ps[:], av_sb[:, c * P:(c + 1) * P], identr[:2, :2],
            is_transpose=True, start=True, stop=True,
        )
        nc.scalar.activation(anT_sb[:, c:c + 1], avT2_ps[:, 1:2], AF.Copy)
    eanloc_sb = ph1_pool.tile([P, sc_n], f32, name="eanloc_sb")
    nc.scalar.activation(eanloc_sb[:], anT_sb[:], AF.Exp, scale=1.0)

    h2an_dram = dram_pool.tile([sr, dout], bf16, name="h2an_dram")
    h2an_pm = h2an_dram[0:s].rearrange("(p kl) d -> kl p d", kl=sc_n)
    for c in range(sc_n):
        hn_ps = tp_psum.tile([P, P], f32, name="hn_ps", tag="tp")
        nc.tensor.matmul(
            hn_ps[:], hT_sb[:, c * P:(c + 1) * P], ident[:],
            is_transpose=True, start=True, stop=True,
        )
        h2c_sb = fin_pool.tile([P, dout], bf16, name="h2c_sb")
        nc.scalar.activation(h2c_sb[:], hn_ps[:], AF.Copy,
                             scale=eanloc_sb[:, c:c + 1])
        nc.sync.dma_start(h2an_pm[c], h2c_sb[:])
    # a_n rows [8, 128] bf16: transpose anT back to node order for the tail
    anTb_sb = ph1_pool.tile([P, sc_n], bf16, name="anTb_sb")
    nc.scalar.activation(anTb_sb[:], anT_sb[:], AF.Copy)
    anb_ps = tp_psum.tile([sc_n, P], bf16, name="anb_ps", tag="tp")
    nc.tensor.matmul(anb_ps[:], anTb_sb[:], identb[:],
                     is_transpose=True, start=True, stop=True)
    anrow_sb = ph1_pool.tile([sc_n, P], bf16, name="anrow_sb")
    nc.scalar.activation(anrow_sb[:], anb_ps[:], AF.Copy)
    nc.sync.dma_start(h2an_dram[s:sr], anrow_sb[:])

    groups = [list(range(NCORES))]
    h2full_dram = dram_pool.tile([NCORES * sr, dout], bf16, addr_space="Shared",
                                 name="h2full")
    nc.gpsimd.collective_compute(
        "AllGather", ALU.bypass, replica_groups=groups,
        ins=[h2an_dram.opt()], outs=[h2full_dram.opt()],
    )

    # ---- Phase 3: unpack gathered a_n -> w2/v2, h2 block tiles -----------
    anf_raw = ph1_pool.tile([jc_n, P], bf16, name="anf_raw")
    for cc in range(NCORES):
        nc.sync.dma_start(
            anf_raw[cc * sc_n:(cc + 1) * sc_n, :],
            h2full_dram[cc * sr + s:(cc + 1) * sr, :],
        )
    anf_ps = tp_psum.tile([P, jc_n], bf16, name="anf_ps", tag="tp")
    nc.tensor.matmul(anf_ps[:], anf_raw[:], identb[:jc_n, :jc_n],
                     is_transpose=True, start=True, stop=True)
    m_sb = const_pool.tile([P, jc_n], f32, name="m_sb")
    nc.scalar.activation(m_sb[:], anf_ps[:], AF.Exp, scale=-0.8)
    ean_sb = const_pool.tile([P, jc_n], bf16, name="ean_sb")
    nc.scalar.activation(ean_sb[:], anf_ps[:], AF.Exp, scale=1.0)

    # gathered h2 as 8 per-source-core tiles (first matmul waits on block 0
    # only); 4 KB per-partition descriptors via the partition-major layout
    h2blk = []
    for cc in range(NCORES):
        hb = ph1_pool.tile([P, sc_n * dout], bf16, name="h2blk", tag=f"h2b{cc}")
        nc.sync.dma_start(
            hb[:],
            h2full_dram[cc * sr:cc * sr + s, :].rearrange(
                "(p kl) d -> p (kl d)", kl=sc_n),
        )
        h2blk.append(hb)

    # adjacency stream (whole slab, ring of 6 super-chunks)
    adj_t = []
    for g in range(g_n):
        at = adj_pool.tile([P, GP * s], bf16, name="adj_t")
        nc.sync.dma_start(
            at[:],
            adjt[g * GP * P:(g + 1) * GP * P, :].rearrange(
                "(p r) i -> p (r i)", r=GP),
        )
        adj_t.append(at)

    # ---- Phase 4: main loop over adj super-chunks (unchanged) ------------
    mm_ps = [acc_psum.tile([P, nb], f32, name=f"mm_ps{b}") for b in range(ib_n)]
    rs_ps = [acc_psum.tile([1, nb], f32, name=f"rs_ps{b}") for b in range(ib_n)]
    for g in range(g_n):
        for r in range(GP):
            j = g * GP + r
            q_t = q_pool.tile([P, s], bf16, name="q_t")
            nc.vector.scalar_tensor_tensor(
                q_t[:], wb_sb[:], m_sb[:, j:j + 1],
                adj_t[g][:, r * s:(r + 1) * s],
                op0=ALU.max, op1=ALU.mult,
            )
            st = h2blk[j // sc_n][:, (j % sc_n) * dout:(j % sc_n + 1) * dout]
            for b in range(ib_n):
                nc.tensor.matmul(
                    mm_ps[b][:], st, q_t[:, b * nb:(b + 1) * nb],
                    start=(j == 0), stop=(j == jc_n - 1),
                )
            for b in range(ib_n):
                nc.tensor.matmul(
                    rs_ps[b][:], ean_sb[:, j:j + 1], q_t[:, b * nb:(b + 1) * nb],
                    start=(j == 0), stop=(j == jc_n - 1),
                )

    # ---- Phase 5: normalize, relu, transpose out -------------------------
    rs_sb = ph1_pool.tile([1, s], f32, name="rs_sb")
    for b in range(ib_n):
        nc.scalar.activation(rs_sb[:, b * nb:(b + 1) * nb], rs_ps[b][:], AF.Copy)
    rs_dram = dram_pool.tile([sc_n, P], f32, name="rs_dram")
    nc.sync.dma_start(rs_dram[:].rearrange("k p -> (k p)")[None, :], rs_sb[0:1, :])
    rs_raw = ph1_pool.tile([sc_n, P], f32, name="rs_raw")
    nc.sync.dma_start(rs_raw[:], rs_dram[:])
    rsT_ps = tp_psum.tile([P, sc_n], f32, name="rsT_ps", tag="tp")
    nc.tensor.matmul(rsT_ps[:], rs_raw[:], ident[:sc_n, :sc_n],
                     is_transpose=True, start=True, stop=True)
    rrT_sb = ph1_pool.tile([P, sc_n], f32, name="rrT_sb")
    nc.vector.reciprocal(rrT_sb[:], rsT_ps[:])

    mo_sb = ph1_pool.tile([P, s], f32, name="mo_sb")
    for b in range(ib_n):
        nc.scalar.activation(mo_sb[:, b * nb:(b + 1) * nb], mm_ps[b][:], AF.Copy)
    for c in range(sc_n):
        ot_ps = tp_psum.tile([P, P], f32, name="ot_ps", tag="tp")
        nc.tensor.matmul(
            ot_ps[:], mo_sb[:, c * P:(c + 1) * P], ident[:],
            is_transpose=True, start=True, stop=True,
        )
        oc_sb = fin_pool.tile([P, dout], f32, name="oc_sb")
        nc.scalar.activation(oc_sb[:], ot_ps[:], AF.Relu, scale=rrT_sb[:, c:c + 1])
        nc.sync.dma_start(out[c * P:(c + 1) * P, :], oc_sb[:])


def build_nc(n=N, s=S, din=DIN, dout=DOUT):
    from contextlib import ExitStack

    import concourse.bacc as bacc
    import concourse.tile as tile

    nc = bacc.Bacc(
        "TRN2",
        target_bir_lowering=False,
        debug=False,
        num_devices=NCORES,
    )
    with tile.TileContext(nc) as tc, ExitStack() as ctx:
        _emit(nc, tc, ctx, n, s, din, dout)
    nc.compile()
    return nc


def prep_adjt(adj_slab):
    """[s, n] adj row-slab -> transposed [n, s] bf16 with GP-row interleave."""
    import ml_dtypes

    adjt = adj_slab.T  # [n, s]
    n, s = adjt.shape
    P = 128
    g = n // (GP * P)
    adjt = adjt.reshape(g, GP, P, s).transpose(0, 2, 1, 3).reshape(n, s)
    return np.ascontiguousarray(adjt.astype(ml_dtypes.bfloat16))


def make_in_maps(x, adj, W, attn_self, attn_neigh, s=S):
    import ml_dtypes

    bf = ml_dtypes.bfloat16
    att = np.concatenate([attn_self, attn_neigh], axis=1).astype(bf)
    wmat = np.ascontiguousarray(W.astype(bf))
    wtt = np.ascontiguousarray(W.T.astype(bf))
    in_maps = []
    for c in range(NCORES):
        sl = slice(c * s, (c + 1) * s)
        in_maps.append({
            "adjt": prep_adjt(adj[sl, :]),
            "xt": np.ascontiguousarray(x[sl, :].T.astype(bf)),
            "wmat": wmat,
            "wt": wtt,
            "att": att,
        })
    return in_maps


def kernel(x, adj, W, attn_self, attn_neigh):
    from concourse.bass_utils import run_bass_kernel_spmd

    x = np.asarray(x, dtype=np.float32)
    adj = np.asarray(adj, dtype=np.float32)
    W = np.asarray(W, dtype=np.float32)
    attn_self = np.asarray(attn_self, dtype=np.float32)
    attn_neigh = np.asarray(attn_neigh, dtype=np.float32)

    nc = build_nc()
    in_maps = make_in_maps(x, adj, W, attn_self, attn_neigh)
    res = run_bass_kernel_spmd(nc, in_maps, list(range(NCORES)))
    return np.concatenate([res.results[c]["out"] for c in range(NCORES)], axis=0)



# revision 10
# speedup vs baseline: 1.2050x; 1.2050x over previous
"""AttentiveGraphConvolution (GAT-style layer) on 8 trn2 NeuronCores.

Math (reference):
    h   = x @ W                       [N, D]
    a_s = h @ attn_self               [N, 1]
    a_n = h @ attn_neigh              [N, 1]
    e   = leaky_relu(a_s + a_n.T, 0.2)
    e   = e + NEG_INF * (1 - adj)
    out = relu(softmax(e, -1) @ h)

Reformulation (exact in fp32 up to rounding):
    s_ij = a_s[i] + a_n[j]
    exp(leaky(s)) = exp(0.2 s) * max(exp(0.8 s), 1)     (alpha = 0.2)
    masked weight t_ij = adj_ij * u2_i * v2_j * max(w_i w2_j, 1)
    out_i = relu( (sum_j q_ji h2_j) / (sum_j q_ji v2_j) )   (u2_i cancels)
    with q_ji = adjT_ji * max(w_i, m_j),  w = e^{0.8 a_s}, m = e^{-0.8 a_n}
         h2_j = v2_j * h_j,  v2 = e^{a_n}

Collective-free design (profile-driven rewrite of the AllGather version):
  * The old kernel's CC barrier + AllGather occupied ~80us of serialized
    wall time (47.5us rendezvous stagger + 21.4us gather + unpack).  Here
    every core computes the FULL h = x @ W itself from a replicated bf16
    copy of x (8 MB extra DMA, ~14us of PE) - zero cross-core traffic, so
    each core's span is its own work and launch stagger costs nothing.
  * The DVE q-op was the baseline's 82us co-bottleneck: scalar_tensor_tensor
    supports no DVE perf modes (1 elem/cycle/lane).  Split into
    tensor_scalar(max) [4x_2p mode] + one batched tensor_tensor(mult)
    [2x_1p mode] per 4-chunk super-tile.
  * Main matmuls use the full 1024-wide bf16 moving operand; LDWEIGHTS
    hides under the previous matmul's stream (PE reorder window).
  * Node order is rotated per-core (own slab first) so a_s/wb and the
    first h2 chunks are ready a few us in; the j-loop, adjacency rows and
    x columns all live in the same rotated space, so no un-permutation is
    needed anywhere (output rows come out in natural order).

Sharding: output rows across 8 cores; adj row-slab shipped transposed
[n, s] bf16 (binary => exact); x replicated [din, n] bf16 in
(node-block, k-chunk) tile order so h production pipelines behind DMA.
"""

import numpy as np

N = 8192
DIN = 512
DOUT = 128
NCORES = 8
S = N // NCORES     # 1024 output rows per core
P = 128
JC = N // P         # 64 j chunks
KC = DIN // P       # 4 contraction chunks
XB = 8              # x node blocks of 1024
SUP = 4             # j chunks per DVE/adj super-tile
GN = JC // SUP      # 16 super tiles


def _emit(nc, tc, ctx, n, s, din, dout):
    from concourse import masks, mybir

    f32 = mybir.dt.float32
    bf16 = mybir.dt.bfloat16
    AF = mybir.ActivationFunctionType
    ALU = mybir.AluOpType

    adjt = nc.dram_tensor("adjt", [n, s], bf16, kind="ExternalInput")
    xt = nc.dram_tensor("xt", [XB * KC * P, P * 8], bf16, kind="ExternalInput")
    wmat = nc.dram_tensor("wmat", [din, dout], bf16, kind="ExternalInput")
    att = nc.dram_tensor("att", [dout, 2], bf16, kind="ExternalInput")
    out = nc.dram_tensor("out", [s, dout], f32, kind="ExternalOutput")

    const_pool = ctx.enter_context(tc.tile_pool(name="const", bufs=1))
    ph1_pool = ctx.enter_context(tc.tile_pool(name="ph1", bufs=1))
    x_pool = ctx.enter_context(tc.tile_pool(name="xp", bufs=4))
    adj_pool = ctx.enter_context(tc.tile_pool(name="adj", bufs=6))
    t1_pool = ctx.enter_context(tc.tile_pool(name="t1", bufs=2))
    q_pool = ctx.enter_context(tc.tile_pool(name="q", bufs=3))
    fin_pool = ctx.enter_context(tc.tile_pool(name="fin", bufs=2))
    dram_pool = ctx.enter_context(tc.tile_pool(name="dram", bufs=1, space="DRAM"))
    ph1_psum = ctx.enter_context(tc.tile_pool(name="ph1_psum", bufs=2, space="PSUM"))
    tp_psum = ctx.enter_context(tc.tile_pool(name="tp_psum", bufs=2, space="PSUM"))
    acc_psum = ctx.enter_context(tc.tile_pool(name="acc_psum", bufs=1, space="PSUM"))

    ident = const_pool.tile([P, P], f32, name="ident")
    masks.make_identity(nc, ident[:])
    identb = const_pool.tile([P, P], bf16, name="identb")
    nc.scalar.activation(identb[:], ident[:], AF.Copy)
    ones1 = const_pool.tile([1, P], bf16, name="ones1")
    nc.gpsimd.memset(ones1[:], 1.0)

    # ---- input DMAs: x blocks first (critical path), adj interleaved -----
    att_sb = const_pool.tile([P, 2], bf16, name="att_sb")
    nc.sync.dma_start(att_sb[:], att[:])
    w_sb = []
    for k in range(KC):
        wk = ph1_pool.tile([P, dout], bf16, name="w_sb", tag=f"w{k}")
        nc.sync.dma_start(wk[:], wmat[k * P:(k + 1) * P, :])
        w_sb.append(wk)

    x_sb = {}

    def dma_x_block(b):
        for k in range(KC):
            t = x_pool.tile([P, 8 * P], bf16, name="x_sb", tag=f"xk{k}")
            nc.sync.dma_start(t[:], xt[(b * KC + k) * P:(b * KC + k + 1) * P, :])
            x_sb[(b, k)] = t

    adj_t = []

    def dma_adj_super(g):
        at = adj_pool.tile([P, SUP * s], bf16, name="adj_t")
        for r in range(SUP):
            j = g * SUP + r
            nc.sync.dma_start(at[:, r * s:(r + 1) * s],
                              adjt[j * P:(j + 1) * P, :])
        adj_t.append(at)

    # priority interleave: x fully in flight early, adj streams behind
    dma_x_block(0)
    dma_x_block(1)
    dma_adj_super(0)
    for b in range(2, XB):
        dma_x_block(b)
        dma_adj_super(b - 1)
    for g in range(XB - 1, GN):
        dma_adj_super(g)

    # ---- phase 1: hT = (x @ W).T, a_s/a_n, per-chunk m/ean scalars -------
    hT_sb = ph1_pool.tile([P, n], bf16, name="hT_sb")
    av_sb = ph1_pool.tile([2, n], f32, name="av_sb")
    m_sb = ph1_pool.tile([P, JC], f32, name="m_sb")
    ean_sb = ph1_pool.tile([P, JC], bf16, name="ean_sb")
    eansc_sb = ph1_pool.tile([P, JC], f32, name="eansc_sb")
    an_dram = dram_pool.tile([JC, P], f32, name="an_dram")

    NB = 512  # psum block for hT production
    for b in range(XB):
        for half in range(2):
            c0 = b * 8 * P + half * NB
            hT_ps = ph1_psum.tile([P, NB], f32, name="hT_ps", tag="ph1")
            for k in range(KC):
                nc.tensor.matmul(
                    hT_ps[:], w_sb[k][:],
                    x_sb[(b, k)][:, half * NB:(half + 1) * NB],
                    start=(k == 0), stop=(k == KC - 1),
                )
            nc.scalar.activation(hT_sb[:, c0:c0 + NB], hT_ps[:], AF.Copy)
        for half in range(2):
            c0 = b * 8 * P + half * NB
            av_ps = ph1_psum.tile([2, NB], f32, name="av_ps", tag="ph1")
            nc.tensor.matmul(av_ps[:], att_sb[:], hT_sb[:, c0:c0 + NB],
                             start=True, stop=True)
            nc.scalar.activation(av_sb[:, c0:c0 + NB], av_ps[:], AF.Copy)
        # a_n for this block -> DRAM round trip -> [128, 8] chunk columns
        nc.sync.dma_start(
            an_dram[b * 8:(b + 1) * 8, :].rearrange("k p -> (k p)")[None, :],
            av_sb[1:2, b * 8 * P:(b + 1) * 8 * P])
        anraw_sb = fin_pool.tile([8, P], f32, name="anraw_sb")
        nc.sync.dma_start(anraw_sb[:], an_dram[b * 8:(b + 1) * 8, :])
        anT_ps = tp_psum.tile([P, 8], f32, name="anT_ps", tag="tp")
        nc.tensor.matmul(anT_ps[:], anraw_sb[:],
                         ident[:8, :8], is_transpose=True,
                         start=True, stop=True)
        nc.scalar.activation(m_sb[:, b * 8:(b + 1) * 8], anT_ps[:],
                             AF.Exp, scale=-0.8)
        nc.scalar.activation(ean_sb[:, b * 8:(b + 1) * 8], anT_ps[:],
                             AF.Exp, scale=1.0)
        nc.scalar.activation(eansc_sb[:, b * 8:(b + 1) * 8], anT_ps[:],
                             AF.Exp, scale=1.0)
        if b == 0:
            # wb[p, i] = exp(0.8 a_s_i) for the local slab (chunks 0..7)
            wrow_sb = ph1_pool.tile([1, s], bf16, name="wrow_sb")
            nc.scalar.activation(wrow_sb[:], av_sb[0:1, 0:s], AF.Exp, scale=0.8)
            wb_sb = ph1_pool.tile([P, s], bf16, name="wb_sb")
            for half in range(2):
                wb_ps = ph1_psum.tile([P, NB], f32, name="wb_ps", tag="ph1")
                nc.tensor.matmul(wb_ps[:], ones1[:],
                                 wrow_sb[:, half * NB:(half + 1) * NB],
                                 start=True, stop=True)
                nc.scalar.activation(wb_sb[:, half * NB:(half + 1) * NB],
                                     wb_ps[:], AF.Copy)

    # ---- h2 chunks: transpose hT, scale by e^{a_n} -----------------------
    h2_sb = ph1_pool.tile([P, n], bf16, name="h2_sb")
    for c in range(JC):
        hn_ps = tp_psum.tile([P, P], bf16, name="hn_ps", tag="tp")
        nc.tensor.matmul(hn_ps[:], hT_sb[:, c * P:(c + 1) * P], identb[:],
                         is_transpose=True, start=True, stop=True)
        nc.scalar.activation(h2_sb[:, c * P:(c + 1) * P], hn_ps[:],
                             AF.Copy, scale=eansc_sb[:, c:c + 1])

    # ---- main loop: q = max(wb, m_j) * adjT, accumulate num/den ----------
    NBM = 512  # matmul psum block (one bank)
    mm_ps = [acc_psum.tile([P, NBM], f32, name=f"mm_ps{b}") for b in range(2)]
    rs_ps = [acc_psum.tile([1, NBM], f32, name=f"rs_ps{b}") for b in range(2)]
    for g in range(GN):
        t1 = t1_pool.tile([P, SUP * s], bf16, name="t1")
        for r in range(SUP):
            j = g * SUP + r
            nc.vector.tensor_scalar_max(t1[:, r * s:(r + 1) * s], wb_sb[:],
                                        m_sb[:, j:j + 1])
        q_t = q_pool.tile([P, SUP * s], bf16, name="q_t")
        nc.vector.tensor_tensor(q_t[:], t1[:], adj_t[g][:], ALU.mult)
        for r in range(SUP):
            j = g * SUP + r
            for b in range(2):
                nc.tensor.matmul(mm_ps[b][:], h2_sb[:, j * P:(j + 1) * P],
                                 q_t[:, r * s + b * NBM:r * s + (b + 1) * NBM],
                                 start=(j == 0), stop=(j == JC - 1))
            for b in range(2):
                nc.tensor.matmul(rs_ps[b][:], ean_sb[:, j:j + 1],
                                 q_t[:, r * s + b * NBM:r * s + (b + 1) * NBM],
                                 start=(j == 0), stop=(j == JC - 1))

    # ---- tail: normalize, relu, transpose out ----------------------------
    rs_sb = ph1_pool.tile([1, s], f32, name="rs_sb")
    for b in range(2):
        nc.scalar.activation(rs_sb[:, b * NBM:(b + 1) * NBM], rs_ps[b][:],
                             AF.Copy)
    rsT_ps = tp_psum.tile([P, 8], f32, name="rsT_ps", tag="tp")
    for c in range(8):
        nc.tensor.matmul(rsT_ps[:, c:c + 1], rs_sb[0:1, c * P:(c + 1) * P],
                         ident[:1, :1], is_transpose=True,
                         start=True, stop=True)
    rrT_sb = ph1_pool.tile([P, 8], f32, name="rrT_sb")
    nc.vector.reciprocal(rrT_sb[:], rsT_ps[:])

    mo_sb = ph1_pool.tile([P, s], f32, name="mo_sb")
    for b in range(2):
        nc.scalar.activation(mo_sb[:, b * NBM:(b + 1) * NBM], mm_ps[b][:],
                             AF.Copy)
    for c in range(8):
        ot_ps = tp_psum.tile([P, P], f32, name="ot_ps", tag="tp")
        nc.tensor.matmul(ot_ps[:], mo_sb[:, c * P:(c + 1) * P], ident[:],
                         is_transpose=True, start=True, stop=True)
        oc_sb = fin_pool.tile([P, dout], f32, name="oc_sb")
        nc.scalar.activation(oc_sb[:], ot_ps[:], AF.Relu,
                             scale=rrT_sb[:, c:c + 1])
        nc.sync.dma_start(out[c * P:(c + 1) * P, :], oc_sb[:])


def build_nc(n=N, s=S, din=DIN, dout=DOUT):
    from contextlib import ExitStack

    import concourse.bacc as bacc
    import concourse.tile as tile

    nc = bacc.Bacc(
        "TRN2",
        target_bir_lowering=False,
        debug=False,
        num_devices=NCORES,
    )
    with tile.TileContext(nc) as tc, ExitStack() as ctx:
        _emit(nc, tc, ctx, n, s, din, dout)
    nc.compile()
    return nc


def make_in_maps(x, adj, W, attn_self, attn_neigh, s=S):
    import ml_dtypes

    bf = ml_dtypes.bfloat16
    att = np.concatenate([attn_self, attn_neigh], axis=1).astype(bf)
    wmat = np.ascontiguousarray(W.astype(bf))
    xb = x.astype(bf)
    adjb = adj.astype(bf)
    in_maps = []
    for c in range(NCORES):
        perm = np.concatenate([np.arange(c * s, N), np.arange(0, c * s)])
        # adjT in rotated j order: [n, s]
        adjt = np.ascontiguousarray(adjb[c * s:(c + 1) * s, :][:, perm].T)
        # x^T in rotated node order, tiled (block, k): [XB*KC*128, 1024]
        xt_r = xb[perm, :].T  # [din, n]
        tiles = []
        for b in range(XB):
            for k in range(KC):
                tiles.append(xt_r[k * P:(k + 1) * P, b * 8 * P:(b + 1) * 8 * P])
        xt = np.ascontiguousarray(np.concatenate(tiles, axis=0))
        in_maps.append({
            "adjt": adjt,
            "xt": xt,
            "wmat": wmat,
            "att": att,
        })
    return in_maps


def kernel(x, adj, W, attn_self, attn_neigh):
    from concourse.bass_utils import run_bass_kernel_spmd

    x = np.asarray(x, dtype=np.float32)
    adj = np.asarray(adj, dtype=np.float32)
    W = np.asarray(W, dtype=np.float32)
    attn_self = np.asarray(attn_self, dtype=np.float32)
    attn_neigh = np.asarray(attn_neigh, dtype=np.float32)

    nc = build_nc()
    in_maps = make_in_maps(x, adj, W, attn_self, attn_neigh)
    res = run_bass_kernel_spmd(nc, in_maps, list(range(NCORES)))
    return np.concatenate([res.results[c]["out"] for c in range(NCORES)], axis=0)


# revision 22
# speedup vs baseline: 1.3531x; 1.1228x over previous
"""AttentiveGraphConvolution (GAT-style layer) on 8 trn2 NeuronCores.

Math (reference):
    h   = x @ W                       [N, D]
    a_s = h @ attn_self               [N, 1]
    a_n = h @ attn_neigh              [N, 1]
    e   = leaky_relu(a_s + a_n.T, 0.2)
    e   = e + NEG_INF * (1 - adj)
    out = relu(softmax(e, -1) @ h)

Reformulation (exact in fp32 up to rounding):
    exp(leaky(s)) = exp(0.2 s) * max(exp(0.8 s), 1),  s_ij = a_s_i + a_n_j
    q2_ji = adjT_ji * max(w_i, m_j) * v2_j            [j, i] layout
            with w = e^{0.8 a_s}, m = e^{-0.8 a_n}, v2 = e^{a_n}
    out_i = relu( (sum_j q2_ji h_j) / (sum_j q2_ji) )  (u2_i cancels)

Collective-free, single-pass design (v2, trace-driven):
  * No AllGather: every core computes the full h = x @ W from a
    replicated bf16 x (the old CC barrier+gather cost ~80us serialized).
  * Few, big DMAs: x ships as 8 x 1MB block tiles, adj as 16 x 1MB
    GP-interleaved supers.  The first version used 125 small DMAs and
    the sync sequencer's ~0.6us/trigger serialized the whole head of
    the kernel.  The a_n round-trip DMAs ride the idle gpsimd queue.
  * v2 = e^{a_n} folds into the DVE tensor_scalar op as its second
    scalar ((wb max m_j) * v2_j), so h chunks need no per-chunk scaled
    copy: 4 plain transposes batch into one [128,512] ACT copy, and the
    denominator weights become a constant ones column (no LDW churn).
  * Phase 1 interleaves with the main loop per x-block (block b ->
    supers 2b, 2b+1), so the PE never idles >1us and the HAM clock
    gate stays at 2.4 GHz; adj super triggers are emitted inside the
    main loop so the 5-buffer adjacency ring never stalls the sync
    queue.
  * DVE: tensor_scalar(max,mult) [4x mode] + one batched 4096-wide
    tensor_tensor(mult) [2x mode] per super (scalar_tensor_tensor has
    no DVE perf mode - that was the baseline's 82us co-bottleneck).
"""

import numpy as np

N = 8192
DIN = 512
DOUT = 128
NCORES = 8
S = N // NCORES     # 1024 output rows per core
P = 128
JC = N // P         # 64 j chunks
KC = DIN // P       # 4 contraction chunks
XB = 8              # x node blocks of 1024
SUP = 4             # j chunks per DVE/adj super-tile
GN = JC // SUP      # 16 super tiles
GP = 4              # adjacency rows per partition line (DMA descriptor size)
ADJ_BUFS = 5


def _emit(nc, tc, ctx, n, s, din, dout):
    from concourse import masks, mybir

    f32 = mybir.dt.float32
    bf16 = mybir.dt.bfloat16
    AF = mybir.ActivationFunctionType
    ALU = mybir.AluOpType

    adjt = nc.dram_tensor("adjt", [n, s], bf16, kind="ExternalInput")
    xt = nc.dram_tensor("xt", [XB * P, KC * 8 * P], bf16, kind="ExternalInput")
    wmat2 = nc.dram_tensor("wmat2", [P, KC * dout], bf16, kind="ExternalInput")
    att = nc.dram_tensor("att", [dout, 2], bf16, kind="ExternalInput")
    out = nc.dram_tensor("out", [s, dout], f32, kind="ExternalOutput")

    const_pool = ctx.enter_context(tc.tile_pool(name="const", bufs=1))
    ph1_pool = ctx.enter_context(tc.tile_pool(name="ph1", bufs=1))
    adj_pool = ctx.enter_context(tc.tile_pool(name="adj", bufs=ADJ_BUFS))
    t1_pool = ctx.enter_context(tc.tile_pool(name="t1", bufs=2))
    q_pool = ctx.enter_context(tc.tile_pool(name="q", bufs=3))
    fin_pool = ctx.enter_context(tc.tile_pool(name="fin", bufs=2))
    dram_pool = ctx.enter_context(tc.tile_pool(name="dram", bufs=1, space="DRAM"))
    ph1_psum = ctx.enter_context(tc.tile_pool(name="ph1_psum", bufs=2, space="PSUM"))
    tp_psum = ctx.enter_context(tc.tile_pool(name="tp_psum", bufs=2, space="PSUM"))
    acc_psum = ctx.enter_context(tc.tile_pool(name="acc_psum", bufs=1, space="PSUM"))

    ident = const_pool.tile([P, P], f32, name="ident")
    masks.make_identity(nc, ident[:])
    identb = const_pool.tile([P, P], bf16, name="identb")
    nc.scalar.activation(identb[:], ident[:], AF.Copy)
    ones1 = const_pool.tile([1, P], bf16, name="ones1")
    nc.gpsimd.memset(ones1[:], 1.0)
    onescol = const_pool.tile([P, 1], bf16, name="onescol")
    nc.gpsimd.memset(onescol[:], 1.0)

    # ---- input DMAs (big, few) ------------------------------------------
    w_sb = const_pool.tile([P, KC * dout], bf16, name="w_sb")
    nc.sync.dma_start(w_sb[:], wmat2[:])
    att_sb = const_pool.tile([P, 2], bf16, name="att_sb")
    nc.sync.dma_start(att_sb[:], att[:])

    x_pool = ctx.enter_context(tc.tile_pool(name="xp", bufs=6))
    x_sb = {}

    def dma_x_block(b):
        t = x_pool.tile([P, KC * 8 * P], bf16, name="x_sb")
        nc.sync.dma_start(t[:], xt[b * P:(b + 1) * P, :])
        x_sb[b] = t

    adj_t = {}

    def dma_adj_super(g):
        at = adj_pool.tile([P, SUP * s], bf16, name="adj_t")
        nc.sync.dma_start(
            at[:],
            adjt[g * SUP * P:(g + 1) * SUP * P, :].rearrange(
                "(p r) i -> p (r i)", r=GP),
        )
        adj_t[g] = at

    dma_adj_super(0)
    dma_adj_super(1)
    for b in range(6):
        dma_x_block(b)

    # ---- persistent phase-1 tiles ---------------------------------------
    avn_pool = ctx.enter_context(tc.tile_pool(name="avn", bufs=2))
    hT_sb = ph1_pool.tile([P, n], bf16, name="hT_sb")
    m_sb = ph1_pool.tile([P, JC], f32, name="m_sb")
    ean_sb = ph1_pool.tile([P, JC], f32, name="ean_sb")
    h_sb = ph1_pool.tile([P, n], bf16, name="h_sb")
    wb_sb = ph1_pool.tile([P, s], bf16, name="wb_sb")
    an_dram = dram_pool.tile([JC, P], f32, name="an_dram")

    NB = 512
    NBM = 512
    mm_ps = [acc_psum.tile([P, NBM], f32, name=f"mm_ps{b}") for b in range(2)]
    rs_ps = [acc_psum.tile([1, NBM], f32, name=f"rs_ps{b}") for b in range(2)]

    def emit_super(g):
        # adjacency ring refill (trigger lands on sync with deps satisfied)
        if 2 <= g + 2 < GN:
            dma_adj_super(g + 2)
        # q2 = (wb max m_j) * ean_j * adjT   (DVE: 4x TS + 2x batched TT)
        t1 = t1_pool.tile([P, SUP * s], bf16, name="t1")
        for r in range(SUP):
            j = g * SUP + r
            nc.vector.tensor_scalar(t1[:, r * s:(r + 1) * s], wb_sb[:],
                                    m_sb[:, j:j + 1], ean_sb[:, j:j + 1],
                                    ALU.max, ALU.mult)
        q_t = q_pool.tile([P, SUP * s], bf16, name="q_t")
        nc.vector.tensor_tensor(q_t[:], t1[:], adj_t[g][:], ALU.mult)
        # h chunks for this super: 4 transposes -> one batched ACT copy
        hn_ps = tp_psum.tile([P, SUP * P], bf16, name="hn_ps", tag="tp")
        for r in range(SUP):
            j = g * SUP + r
            nc.tensor.matmul(hn_ps[:, r * P:(r + 1) * P],
                             hT_sb[:, j * P:(j + 1) * P], identb[:],
                             is_transpose=True, start=True, stop=True)
        c0 = g * SUP * P
        nc.scalar.activation(h_sb[:, c0:c0 + SUP * P], hn_ps[:], AF.Copy)
        # accumulate numerator / denominator
        for r in range(SUP):
            j = g * SUP + r
            for b in range(2):
                nc.tensor.matmul(mm_ps[b][:], h_sb[:, j * P:(j + 1) * P],
                                 q_t[:, r * s + b * NBM:r * s + (b + 1) * NBM],
                                 start=(j == 0), stop=(j == JC - 1))
            for b in range(2):
                nc.tensor.matmul(rs_ps[b][:], onescol[:],
                                 q_t[:, r * s + b * NBM:r * s + (b + 1) * NBM],
                                 start=(j == 0), stop=(j == JC - 1))

    # ---- phase 1 (per x block) interleaved with the main loop -----------
    for b in range(XB):
        if b + 6 < XB:
            dma_x_block(b + 6)
        xb = x_sb[b]
        avn_sb = avn_pool.tile([2, 8 * P], f32, name="avn_sb")
        # hT[:, block] = (x @ W).T
        for half in range(2):
            c0 = b * 8 * P + half * NB
            hT_ps = ph1_psum.tile([P, NB], f32, name="hT_ps", tag="ph1")
            for k in range(KC):
                nc.tensor.matmul(
                    hT_ps[:], w_sb[:, k * dout:(k + 1) * dout],
                    xb[:, k * 8 * P + half * NB:k * 8 * P + (half + 1) * NB],
                    start=(k == 0), stop=(k == KC - 1),
                )
            nc.scalar.activation(hT_sb[:, c0:c0 + NB], hT_ps[:], AF.Copy)
        # a_s / a_n rows for the block
        for half in range(2):
            c0 = b * 8 * P + half * NB
            av_ps = ph1_psum.tile([2, NB], f32, name="av_ps", tag="ph1")
            nc.tensor.matmul(av_ps[:], att_sb[:], hT_sb[:, c0:c0 + NB],
                             start=True, stop=True)
            nc.scalar.activation(avn_sb[:, half * NB:(half + 1) * NB],
                                 av_ps[:], AF.Copy)
        # a_n -> [128, 8] chunk columns via DRAM round trip (gpsimd queue)
        nc.gpsimd.dma_start(
            an_dram[b * 8:(b + 1) * 8, :].rearrange("k p -> (k p)")[None, :],
            avn_sb[1:2, :])
        anraw_sb = fin_pool.tile([8, P], f32, name="anraw_sb")
        nc.gpsimd.dma_start(anraw_sb[:], an_dram[b * 8:(b + 1) * 8, :])
        anT_ps = tp_psum.tile([P, 8], f32, name="anT_ps", tag="tp")
        nc.tensor.matmul(anT_ps[:], anraw_sb[:], ident[:8, :8],
                         is_transpose=True, start=True, stop=True)
        nc.scalar.activation(m_sb[:, b * 8:(b + 1) * 8], anT_ps[:],
                             AF.Exp, scale=-0.8)
        nc.scalar.activation(ean_sb[:, b * 8:(b + 1) * 8], anT_ps[:],
                             AF.Exp, scale=1.0)
        if b == 0:
            # wb[p, i] = exp(0.8 a_s_i) for the local slab (chunks 0..7)
            wrow_sb = ph1_pool.tile([1, s], bf16, name="wrow_sb")
            nc.scalar.activation(wrow_sb[:], avn_sb[0:1, :], AF.Exp, scale=0.8)
            for half in range(2):
                wb_ps = ph1_psum.tile([P, NB], f32, name="wb_ps", tag="ph1")
                nc.tensor.matmul(wb_ps[:], ones1[:],
                                 wrow_sb[:, half * NB:(half + 1) * NB],
                                 start=True, stop=True)
                nc.scalar.activation(wb_sb[:, half * NB:(half + 1) * NB],
                                     wb_ps[:], AF.Copy)
        emit_super(2 * b)
        emit_super(2 * b + 1)

    # ---- tail: normalize, relu, transpose out ----------------------------
    rs_sb = ph1_pool.tile([1, s], f32, name="rs_sb")
    for b in range(2):
        nc.scalar.activation(rs_sb[:, b * NBM:(b + 1) * NBM], rs_ps[b][:],
                             AF.Copy)
    rsT_ps = tp_psum.tile([P, 8], f32, name="rsT_ps", tag="tp")
    for c in range(8):
        nc.tensor.matmul(rsT_ps[:, c:c + 1], rs_sb[0:1, c * P:(c + 1) * P],
                         ident[:1, :1], is_transpose=True,
                         start=True, stop=True)
    rrT_sb = ph1_pool.tile([P, 8], f32, name="rrT_sb")
    nc.vector.reciprocal(rrT_sb[:], rsT_ps[:])

    mo_sb = ph1_pool.tile([P, s], f32, name="mo_sb")
    for b in range(2):
        nc.scalar.activation(mo_sb[:, b * NBM:(b + 1) * NBM], mm_ps[b][:],
                             AF.Copy)
    for c in range(8):
        ot_ps = tp_psum.tile([P, P], f32, name="ot_ps", tag="tp")
        nc.tensor.matmul(ot_ps[:], mo_sb[:, c * P:(c + 1) * P], ident[:],
                         is_transpose=True, start=True, stop=True)
        oc_sb = fin_pool.tile([P, dout], f32, name="oc_sb")
        nc.scalar.activation(oc_sb[:], ot_ps[:], AF.Relu,
                             scale=rrT_sb[:, c:c + 1])
        nc.sync.dma_start(out[c * P:(c + 1) * P, :], oc_sb[:])


def build_nc(n=N, s=S, din=DIN, dout=DOUT):
    from contextlib import ExitStack

    import concourse.bacc as bacc
    import concourse.tile as tile

    nc = bacc.Bacc(
        "TRN2",
        target_bir_lowering=False,
        debug=False,
        num_devices=NCORES,
    )
    with tile.TileContext(nc) as tc, ExitStack() as ctx:
        _emit(nc, tc, ctx, n, s, din, dout)
    nc.compile()
    return nc


def prep_adjt(adj_slab_t):
    """[n, s] transposed adj slab -> GP-row-interleaved layout."""
    n, s = adj_slab_t.shape
    g = n // (GP * P)
    return np.ascontiguousarray(
        adj_slab_t.reshape(g, GP, P, s).transpose(0, 2, 1, 3).reshape(n, s))


def make_in_maps(x, adj, W, attn_self, attn_neigh, s=S):
    import ml_dtypes

    bf = ml_dtypes.bfloat16
    att = np.concatenate([attn_self, attn_neigh], axis=1).astype(bf)
    wmat2 = np.ascontiguousarray(
        np.concatenate([W[k * P:(k + 1) * P, :] for k in range(KC)],
                       axis=1).astype(bf))
    xb = x.astype(bf)
    adjb = adj.astype(bf)
    in_maps = []
    for c in range(NCORES):
        perm = np.concatenate([np.arange(c * s, N), np.arange(0, c * s)])
        adjt = prep_adjt(np.ascontiguousarray(adjb[c * s:(c + 1) * s, :][:, perm].T))
        xt_r = xb[perm, :].T  # [din, n] rotated
        blocks = []
        for b in range(XB):
            blocks.append(np.concatenate(
                [xt_r[k * P:(k + 1) * P, b * 8 * P:(b + 1) * 8 * P]
                 for k in range(KC)], axis=1))
        xt = np.ascontiguousarray(np.concatenate(blocks, axis=0))
        in_maps.append({
            "adjt": adjt,
            "xt": xt,
            "wmat2": wmat2,
            "att": att,
        })
    return in_maps


def kernel(x, adj, W, attn_self, attn_neigh):
    from concourse.bass_utils import run_bass_kernel_spmd

    x = np.asarray(x, dtype=np.float32)
    adj = np.asarray(adj, dtype=np.float32)
    W = np.asarray(W, dtype=np.float32)
    attn_self = np.asarray(attn_self, dtype=np.float32)
    attn_neigh = np.asarray(attn_neigh, dtype=np.float32)

    nc = build_nc()
    in_maps = make_in_maps(x, adj, W, attn_self, attn_neigh)
    res = run_bass_kernel_spmd(nc, in_maps, list(range(NCORES)))
    return np.concatenate([res.results[c]["out"] for c in range(NCORES)], axis=0)
